# revision 3
# baseline (speedup 1.0000x reference)
"""CenterLoss update kernel for Trainium2, 8-core SPMD — class-sharded.

Reference computation (N=16384 samples, C=10000 classes, D=128 dims):
    embeded_labels = labels @ center          # [N,D] gather via one-hot
    diff = embeded_labels - embeded_preds
    grad = (labels.T @ diff) / (counts + 1)   # counts = labels.T @ ones
    out  = center - 0.5 * grad

Because each row of ``labels`` is one-hot, ``labels.T @ labels == diag(counts)``,
so the whole thing collapses to a single pass over ``labels``:

    S      = labels.T @ embeded_preds         # [C,D] per-class sum of preds
    counts = column sums of labels            # [C]
    out    = beta * center + gamma * S
             beta  = 1 - 0.5*counts/(counts+1)
             gamma = 0.5/(counts+1)

Sharding: classes (columns of labels) are sharded across the 8 cores.  Each
core streams its own [N, C/8] column block of labels through the PE exactly
once as the moving matmul operand, accumulating S.T = preds.T @ labels in a
single PSUM region over all 128 k-tiles.  Every core computes its C/8 output
shard entirely locally — no inter-core collective at all.

bf16 reinterpretation trick: the one-hot labels contain only the fp32 bit
patterns 0x00000000 and 0x3F800000.  Viewed as little-endian bf16 pairs these
are (0.0, 0.0) and (0.0, 1.0) — i.e. the HIGH half of every fp32 one-hot
value IS the exact bf16 one-hot value.  So the raw label bytes are matmul'd
directly as bf16 with a stride-2 column view (no conversion, no extra
traffic), and preds is cast to bf16 once on the scalar engine.  That halves
PE time vs the fp32 LOW_HIGH two-pass mode, taking the PE off the critical
path so the DMA stream alone binds.

k-tiles are "virtual": tile q covers sample rows {q + 128*p}.  Labels are
streamed four k-tiles per DMA: viewing labels as [N/4, 4*C/8], quad-tile qq
is the strided row slice labels4[qq::32, :], which makes every DMA descriptor
a contiguous 20 KB partition line — the bigger the per-descriptor payload,
the less per-packet overhead the 16 SDMA engines pay (measured ~49ns/packet
+ bytes/27.9GB/s per engine).  Preds loads as 8 chunks of 8 KB lines
interleaved just ahead of first use, staged f32 then cast to bf16.

Counts: per-partition partial counts accumulate as exact small integers in
bf16 on two engines in parallel (DVE takes quad halves h0/h1, GpSimd h2/h3),
reduced by one ones-matmul pass at quad 30 (hidden under the stream tail);
only the last quad's counts (folded by one DVE add) remain for the end,
reduced chunk-by-chunk so the count->beta/gamma->output chain pipelines.
"""

import numpy as np

N, C, D = 16384, 10000, 128
NCORES = 8
CS = C // NCORES   # 1250 classes per core
LR = 0.5
P = 128
KT = N // P        # 128 virtual k-tiles
NQUAD = KT // 4    # 32 quad-tiles
LATE_Q = NQUAD - 1  # last quad handled by the late count accumulator
NT3 = (CS + P - 1) // P  # output tiles over the class shard
PCW = 2048               # preds chunk width (cols of the [128, N] natural view)
NPCH = (KT * D) // PCW   # 8 chunks; chunk c covers k-tiles [16c, 16c+16)


def _chunks(width, step=512):
    out = []
    c0 = 0
    while c0 < width:
        out.append((c0, min(step, width - c0)))
        c0 += step
    return out


def build_program(cs=CS, d=D, kt=KT):
    """Build the SPMD Bass program (identical on every core)."""
    import concourse.bacc as bacc
    import concourse.mybir as mybir
    import concourse.tile as tile
    from concourse.masks import make_identity

    f32 = mybir.dt.float32
    f32r = mybir.dt.float32r
    bf16 = mybir.dt.bfloat16
    mult = mybir.AluOpType.mult
    add = mybir.AluOpType.add

    n = kt * P
    nt3 = NT3
    assert cs * 4 <= 3 * 2048, "S.T PSUM tile must fit in 3 banks"

    nc = bacc.Bacc(
        "TRN2",
        target_bir_lowering=False,
        debug=False,
        num_devices=NCORES,
    )

    # preds in its natural [128, n] row-major view: partition p holds rows
    # [128p, 128p+128); column block [128q, 128q+128) is then exactly the
    # stationary tile for virtual k-tile q (rows 128p+q on partition p).
    preds = nc.dram_tensor("preds", [P, kt * d], f32, kind="ExternalInput").ap()
    # labels quad view: row r = label rows 4r..4r+3; quad-tile qq is
    # labels4[qq::32, :] (20 KB contiguous per partition line)
    labels4 = nc.dram_tensor(
        "labels", [n // 4, 4 * cs], f32r, kind="ExternalInput"
    ).ap()
    # center arrives host-permuted: element [p, tt*d + j] = center[tt*P + p, j]
    center = nc.dram_tensor("center", [P, nt3 * d], f32, kind="ExternalInput").ap()
    # out leaves in the same permuted layout; host un-permutes
    out = nc.dram_tensor("out", [P, nt3 * d], f32, kind="ExternalOutput").ap()

    # preds chunk c is needed by k-tile 16c = quad 4c; issue a few quads early
    trigger_qq = {}
    for cch in range(NPCH):
        trigger_qq.setdefault(max(0, 4 * cch - 3), []).append(cch)

    with tile.TileContext(nc) as tc:
        with tc.tile_pool(name="const", bufs=1) as const_pool:
            identity = const_pool.tile([P, P], f32, name="identity")
            make_identity(nc, identity[:])
            ones_bf = const_pool.tile([P, 1], bf16, name="ones_bf")
            nc.vector.memset(ones_bf[:], 1.0)

            # center shard in class-on-partition layout, one 5KB-line DMA
            ctr_sb = const_pool.tile([P, nt3 * d], f32, name="ctr_sb")
            nc.gpsimd.dma_start(out=ctr_sb[:], in_=center[:])

            preds_bf = [
                const_pool.tile([P, PCW], bf16, name=f"preds_bf_{cch}")
                for cch in range(NPCH)
            ]

            # per-partition partial counts, exact small integers in bf16.
            # DVE accumulates quad halves h0/h1 (acc_v), GpSimd h2/h3 (acc_g);
            # the last quad goes to acc_vl/acc_gl so only its reduction is
            # left after the stream ends.
            acc_v = const_pool.tile([P, 2 * cs], bf16, name="acc_v")
            acc_g = const_pool.tile([P, 2 * cs], bf16, name="acc_g")
            acc_vl = const_pool.tile([P, 2 * cs], bf16, name="acc_vl")
            acc_gl = const_pool.tile([P, 2 * cs], bf16, name="acc_gl")

            st_sb = const_pool.tile([d, cs], f32, name="st_sb")
            cnt_row = const_pool.tile([1, cs], f32, name="cnt_row")
            o1_all = const_pool.tile([P, nt3 * d], f32, name="o1_all")
            ou_all = const_pool.tile([P, nt3 * d], f32, name="ou_all")
            # last class tile is ragged (98 rows); zero the garbage rows the
            # final full-tile DMA would otherwise read uninitialized
            nc.gpsimd.memset(ou_all[:], 0.0)

            # ---------------- phase 1: stream labels ----------------
            with (
                tc.tile_pool(name="stg", bufs=2) as stg_pool,
                tc.tile_pool(name="lab", bufs=4) as lab_pool,
                tc.tile_pool(name="psum1", bufs=1, space="PSUM") as psum1,
            ):
                st_psum = psum1.tile([d, cs], f32, name="st_psum", space="PSUM")
                cnt_psum = psum1.tile([1, cs], f32, name="cnt_psum", space="PSUM")
                for qq in range(NQUAD):
                    for cch in trigger_qq.get(qq, []):
                        stg = stg_pool.tile(
                            [P, PCW], f32, name=f"stg_{cch}", tag="stg"
                        )
                        peng = nc.sync if cch % 2 == 0 else nc.scalar
                        peng.dma_start(
                            out=stg[:],
                            in_=preds[:, cch * PCW:(cch + 1) * PCW],
                        )
                        nc.scalar.copy(out=preds_bf[cch][:], in_=stg[:])
                    lab = lab_pool.tile(
                        [P, 4 * cs], f32r, name=f"lab_{qq}", tag="lab"
                    )
                    eng = nc.sync if qq % 2 == 0 else nc.scalar
                    eng.dma_start(out=lab[:], in_=labels4[qq::NQUAD, :])
                    # bf16 pair view: element [p, c, 1] is the high half of
                    # fp32 element [p, c] — the exact one-hot value in bf16
                    lab_bf = lab[:].bitcast(bf16).rearrange(
                        "p (c two) -> p c two", two=2
                    )
                    for h in range(4):
                        q = 4 * qq + h
                        cch = q // 16
                        kcol = (q - 16 * cch) * d
                        for c0, w in _chunks(cs):
                            nc.tensor.matmul(
                                out=st_psum[:, c0:c0 + w],
                                lhsT=preds_bf[cch][:, kcol:kcol + d],
                                rhs=lab_bf[:, h * cs + c0:h * cs + c0 + w, 1],
                                start=(q == 0),
                                stop=(q == kt - 1),
                            )
                    lo = lab_bf[:, 0:2 * cs, 1]
                    hi = lab_bf[:, 2 * cs:4 * cs, 1]
                    if qq == 0:
                        nc.vector.tensor_copy(out=acc_v[:], in_=lo)
                        nc.gpsimd.tensor_copy(out=acc_g[:], in_=hi)
                    elif qq < LATE_Q:
                        nc.vector.tensor_add(out=acc_v[:], in0=acc_v[:], in1=lo)
                        nc.gpsimd.tensor_tensor(
                            out=acc_g[:], in0=acc_g[:], in1=hi, op=add
                        )
                    else:
                        nc.vector.tensor_copy(out=acc_vl[:], in_=lo)
                        nc.gpsimd.tensor_copy(out=acc_gl[:], in_=hi)
                    if qq == LATE_Q - 1:
                        # acc_v/acc_g final; their count reduction (both
                        # class-copies of both accs per chunk region) hides
                        # under the tail of the stream
                        for c0, w in _chunks(cs):
                            for ai, acc in enumerate((acc_v, acc_g)):
                                for half in (0, 1):
                                    nc.tensor.matmul(
                                        out=cnt_psum[0:1, c0:c0 + w],
                                        lhsT=ones_bf[:],
                                        rhs=acc[:, half * cs + c0:
                                                half * cs + c0 + w],
                                        start=(ai == 0 and half == 0),
                                        stop=False,
                                    )

                # fold the two last-quad accumulators, then close out each
                # count chunk region and copy it to SBUF immediately so the
                # beta/gamma chain starts while later chunks still reduce
                nc.vector.tensor_add(
                    out=acc_vl[:], in0=acc_vl[:], in1=acc_gl[:]
                )
                for c0, w in _chunks(cs):
                    for half in (0, 1):
                        nc.tensor.matmul(
                            out=cnt_psum[0:1, c0:c0 + w],
                            lhsT=ones_bf[:],
                            rhs=acc_vl[:, half * cs + c0:half * cs + c0 + w],
                            start=False,
                            stop=(half == 1),
                        )
                    nc.scalar.copy(
                        out=cnt_row[0:1, c0:c0 + w],
                        in_=cnt_psum[0:1, c0:c0 + w],
                    )
                nc.scalar.copy(out=st_sb[:], in_=st_psum[:])

            # ---------------- phase 3: elementwise update, all local -------
            # counts for all nt3 class tiles land as columns of one [P, nt3]
            # PSUM tile, so beta/gamma come from 5 batched DVE ops; the
            # beta*center products run per-tile on the scalar engine in
            # parallel with the PE transposes of S.T.
            with (
                tc.tile_pool(name="p3", bufs=2) as p3,
                tc.tile_pool(name="psum3", bufs=1, space="PSUM") as psum3,
            ):
                cnt_all = psum3.tile([P, nt3], f32, name="cnt_all", space="PSUM")
                for tt in range(nt3):
                    w = min(P, cs - tt * P)
                    nc.tensor.transpose(
                        out=cnt_all[0:w, tt:tt + 1],
                        in_=cnt_row[0:1, tt * P:tt * P + w],
                        identity=identity[0:1, 0:1],
                    )
                den = p3.tile([P, nt3], f32, name="den", tag="den", bufs=1)
                nc.vector.tensor_scalar_add(out=den[:], in0=cnt_all[:], scalar1=1.0)
                rec = p3.tile([P, nt3], f32, name="rec", tag="rec", bufs=1)
                nc.vector.reciprocal(out=rec[:], in_=den[:])
                gam = p3.tile([P, nt3], f32, name="gam", tag="gam", bufs=1)
                nc.vector.tensor_scalar_mul(out=gam[:], in0=rec[:], scalar1=0.5)
                bet = p3.tile([P, nt3], f32, name="bet", tag="bet", bufs=1)
                nc.vector.tensor_tensor(
                    out=bet[:], in0=cnt_all[:], in1=rec[:], op=mult
                )
                nc.vector.tensor_scalar(
                    out=bet[:], in0=bet[:],
                    scalar1=-0.5, scalar2=1.0, op0=mult, op1=add,
                )

                for tt in range(nt3):
                    w = min(P, cs - tt * P)
                    # o1 = beta * center on the scalar engine (per-partition
                    # scale), overlapping the PE transpose of the S.T tile
                    nc.scalar.mul(
                        out=o1_all[0:w, tt * d:tt * d + d],
                        in_=ctr_sb[0:w, tt * d:tt * d + d],
                        mul=bet[0:w, tt:tt + 1],
                    )
                    trp = psum3.tile([P, d], f32, name=f"trp_{tt}", tag="trp",
                                     bufs=4, space="PSUM")
                    nc.tensor.transpose(
                        out=trp[0:w, 0:d],
                        in_=st_sb[:, tt * P:tt * P + w],
                        identity=identity[:, 0:d],
                    )
                    nc.vector.scalar_tensor_tensor(
                        out=ou_all[0:w, tt * d:tt * d + d], in0=trp[0:w, 0:d],
                        scalar=gam[0:w, tt:tt + 1],
                        in1=o1_all[0:w, tt * d:tt * d + d], op0=mult, op1=add,
                    )
                    if tt == nt3 // 2 - 1:
                        # first half of the shard is final: overlap its store
                        nc.sync.dma_start(
                            out=out[:, 0:(nt3 // 2) * d],
                            in_=ou_all[:, 0:(nt3 // 2) * d],
                        )
                nc.scalar.dma_start(
                    out=out[:, (nt3 // 2) * d:nt3 * d],
                    in_=ou_all[:, (nt3 // 2) * d:nt3 * d],
                )

    nc.compile()
    return nc


_PROGRAM = None
LAST_RESULTS = None  # BassKernelResults from the most recent run (for test.py)


def _get_program():
    global _PROGRAM
    if _PROGRAM is None:
        _PROGRAM = build_program()
    return _PROGRAM


def kernel(embeded_preds, labels, center):
    from concourse.bass_utils import run_bass_kernel_spmd

    global LAST_RESULTS
    preds = np.ascontiguousarray(np.asarray(embeded_preds, dtype=np.float32))
    lab = np.ascontiguousarray(np.asarray(labels, dtype=np.float32))
    ctr = np.ascontiguousarray(np.asarray(center, dtype=np.float32))
    assert preds.shape == (N, D) and lab.shape == (N, C) and ctr.shape == (C, D)

    nc = _get_program()
    preds_nat = preds.reshape(P, KT * D)  # free view; bytes unchanged

    def permute_center(cj):
        # [cs, d] -> [P, nt3*d] with [p, tt*d + j] = cj[tt*P + p, j]
        cpad = np.zeros((NT3 * P, D), dtype=np.float32)
        cpad[:cj.shape[0]] = cj
        return np.ascontiguousarray(
            cpad.reshape(NT3, P, D).transpose(1, 0, 2).reshape(P, NT3 * D)
        )

    in_maps = [
        {
            "preds": preds_nat,
            "labels": np.ascontiguousarray(lab[:, j * CS:(j + 1) * CS])
                .reshape(N // 4, 4 * CS),
            "center": permute_center(ctr[j * CS:(j + 1) * CS]),
        }
        for j in range(NCORES)
    ]
    res = run_bass_kernel_spmd(nc, in_maps, core_ids=list(range(NCORES)))
    LAST_RESULTS = res

    def unpermute_out(oj):
        # [P, nt3*d] -> [cs, d]: inverse of permute_center
        return oj.reshape(P, NT3, D).transpose(1, 0, 2).reshape(NT3 * P, D)[:CS]

    return np.concatenate(
        [unpermute_out(res.results[j]["out"]) for j in range(NCORES)], axis=0
    )


# revision 4
# speedup vs baseline: 1.0102x; 1.0102x over previous
"""CenterLoss update kernel for Trainium2, 8-core SPMD — class-sharded.

Reference computation (N=16384 samples, C=10000 classes, D=128 dims):
    embeded_labels = labels @ center          # [N,D] gather via one-hot
    diff = embeded_labels - embeded_preds
    grad = (labels.T @ diff) / (counts + 1)   # counts = labels.T @ ones
    out  = center - 0.5 * grad

Because each row of ``labels`` is one-hot, ``labels.T @ labels == diag(counts)``,
so the whole thing collapses to a single pass over ``labels``:

    S      = labels.T @ embeded_preds         # [C,D] per-class sum of preds
    counts = column sums of labels            # [C]
    out    = beta * center + gamma * S
             beta  = 1 - 0.5*counts/(counts+1)
             gamma = 0.5/(counts+1)

Sharding: classes (columns of labels) are sharded across the 8 cores.  Each
core streams its own [N, C/8] column block of labels through the PE exactly
once as the moving matmul operand, accumulating S.T = preds.T @ labels in a
single PSUM region over all 128 k-tiles.  Every core computes its C/8 output
shard entirely locally — no inter-core collective at all.

The entire label stream rides the SWDGE (gpsimd) queue as f32 -> bf16
CASTING DMAs: HBM read traffic is unchanged (the fp32 bytes are read once),
but the SDMA datapath downconverts in flight, so SBUF receives dense bf16
tiles.  One-hot values are exact in bf16, and preds (also cast-DMA'd to bf16
once) only loses ~0.2% — well within tolerance.  Dense bf16 operands make
the PE matmul single-pass at 1 cycle/column (vs the fp32 LOW_HIGH two-pass
mode that co-saturated the PE with the DMA stream), and make the count
accumulation cheap enough for the DVE alone.

k-tiles are "virtual": tile q covers sample rows {q + 128*p}.  With that row
order the stationary preds tiles are contiguous column slices of preds
viewed as [128, N*D/128] row-major.  Labels stream two k-tiles per DMA:
viewing labels as [N/2, 2*C/8], pair-tile qq is the strided row slice
labels2[qq::64, :], making every DMA descriptor a contiguous 10 KB DRAM
read — measured to be the per-engine sweet spot (~24.6 GB/s/engine; 20 KB
and 8 KB lines are both worse).

Counts accumulate on the DVE as exact small integers in bf16: acc_a covers
pair-tiles < 62 and is reduced by single-pass ones-matmuls hidden under the
stream tail; acc_c covers the last 2 pairs, so after the last tile only its
reduction remains, closed chunk-by-chunk so the counts->beta/gamma->output
chain pipelines.  beta*center runs split across the scalar engine and
gpsimd, overlapping the PE transposes of S.T; the output leaves in the same
host-permuted [128, nt3*d] layout center arrives in (5 KB lines, no 512 B
descriptor storm), un-permuted on the host.
"""

import numpy as np

N, C, D = 16384, 10000, 128
NCORES = 8
CS = C // NCORES   # 1250 classes per core
LR = 0.5
P = 128
KT = N // P        # 128 virtual k-tiles
NPAIR = KT // 2    # 64 pair-tiles
LATE_Q = NPAIR - 2  # pairs >= this go to the late count accumulator
NT3 = (CS + P - 1) // P  # output tiles over the class shard
PCW = 2048               # preds chunk width (cols of the [128, N] natural view)
NPCH = (KT * D) // PCW   # 8 chunks; chunk c covers k-tiles [16c, 16c+16)


def _chunks(width, step=512):
    out = []
    c0 = 0
    while c0 < width:
        out.append((c0, min(step, width - c0)))
        c0 += step
    return out


def build_program(cs=CS, d=D, kt=KT):
    """Build the SPMD Bass program (identical on every core)."""
    import concourse.bacc as bacc
    import concourse.mybir as mybir
    import concourse.tile as tile
    from concourse.masks import make_identity

    f32 = mybir.dt.float32
    bf16 = mybir.dt.bfloat16
    mult = mybir.AluOpType.mult
    add = mybir.AluOpType.add

    n = kt * P
    nt3 = NT3
    npair = NPAIR
    assert cs * 4 <= 3 * 2048, "S.T PSUM tile must fit in 3 banks"

    nc = bacc.Bacc(
        "TRN2",
        target_bir_lowering=False,
        debug=False,
        num_devices=NCORES,
    )

    # preds in its natural [128, n] row-major view: partition p holds rows
    # [128p, 128p+128); column block [128q, 128q+128) is then exactly the
    # stationary tile for virtual k-tile q (rows 128p+q on partition p).
    preds = nc.dram_tensor("preds", [P, kt * d], f32, kind="ExternalInput").ap()
    # labels pair view: row r = label rows 2r, 2r+1; pair-tile qq is
    # labels2[qq::64, :] (10 KB contiguous per partition line)
    labels2 = nc.dram_tensor(
        "labels", [n // 2, 2 * cs], f32, kind="ExternalInput"
    ).ap()
    # center arrives host-permuted: element [p, tt*d + j] = center[tt*P + p, j]
    center = nc.dram_tensor("center", [P, nt3 * d], f32, kind="ExternalInput").ap()
    # out leaves in the same permuted layout; host un-permutes
    out = nc.dram_tensor("out", [P, nt3 * d], f32, kind="ExternalOutput").ap()

    # preds chunk c is needed by k-tile 16c = pair 8c; issue a couple early
    trigger_qq = {}
    for cch in range(NPCH):
        trigger_qq.setdefault(max(0, 8 * cch - 2), []).append(cch)

    with tile.TileContext(nc) as tc:
        with tc.tile_pool(name="const", bufs=1) as const_pool:
            identity = const_pool.tile([P, P], f32, name="identity")
            make_identity(nc, identity[:])
            ones_bf = const_pool.tile([P, 1], bf16, name="ones_bf")
            nc.vector.memset(ones_bf[:], 1.0)

            # center shard in class-on-partition layout, one 5KB-line DMA
            ctr_sb = const_pool.tile([P, nt3 * d], f32, name="ctr_sb")
            nc.gpsimd.dma_start(out=ctr_sb[:], in_=center[:])

            preds_bf = [
                const_pool.tile([P, PCW], bf16, name=f"preds_bf_{cch}")
                for cch in range(NPCH)
            ]

            # per-partition partial counts, exact small integers in bf16
            acc_a = const_pool.tile([P, 2 * cs], bf16, name="acc_a")
            acc_c = const_pool.tile([P, 2 * cs], bf16, name="acc_c")

            st_sb = const_pool.tile([d, cs], f32, name="st_sb")
            cnt_row = const_pool.tile([1, cs], f32, name="cnt_row")
            o1_all = const_pool.tile([P, nt3 * d], f32, name="o1_all")
            ou_all = const_pool.tile([P, nt3 * d], f32, name="ou_all")
            # last class tile is ragged (98 rows); zero the garbage rows the
            # final full-tile DMA would otherwise read uninitialized
            nc.vector.memset(ou_all[:], 0.0)

            # ---------------- phase 1: stream labels ----------------
            with (
                tc.tile_pool(name="lab", bufs=6) as lab_pool,
                tc.tile_pool(name="psum1", bufs=1, space="PSUM") as psum1,
            ):
                st_psum = psum1.tile([d, cs], f32, name="st_psum", space="PSUM")
                cnt_psum = psum1.tile([1, cs], f32, name="cnt_psum", space="PSUM")
                for qq in range(npair):
                    for cch in trigger_qq.get(qq, []):
                        nc.gpsimd.dma_start(
                            out=preds_bf[cch][:],
                            in_=preds[:, cch * PCW:(cch + 1) * PCW],
                        )
                    lab = lab_pool.tile(
                        [P, 2 * cs], bf16, name=f"lab_{qq}", tag="lab"
                    )
                    nc.gpsimd.dma_start(out=lab[:], in_=labels2[qq::npair, :])
                    for h in (0, 1):
                        q = 2 * qq + h
                        cch = q // 16
                        kcol = (q - 16 * cch) * d
                        for c0, w in _chunks(cs):
                            nc.tensor.matmul(
                                out=st_psum[:, c0:c0 + w],
                                lhsT=preds_bf[cch][:, kcol:kcol + d],
                                rhs=lab[:, h * cs + c0:h * cs + c0 + w],
                                start=(q == 0),
                                stop=(q == kt - 1),
                            )
                    acc = acc_a if qq < LATE_Q else acc_c
                    if qq in (0, LATE_Q):
                        nc.vector.tensor_copy(out=acc[:], in_=lab[:])
                    else:
                        nc.vector.tensor_add(out=acc[:], in0=acc[:], in1=lab[:])
                    if qq == LATE_Q - 1:
                        # acc_a is final; its count reduction (both class
                        # copies into the same PSUM region) hides under the
                        # tail of the stream
                        for c0, w in _chunks(cs):
                            for half in (0, 1):
                                nc.tensor.matmul(
                                    out=cnt_psum[0:1, c0:c0 + w],
                                    lhsT=ones_bf[:],
                                    rhs=acc_a[:, half * cs + c0:
                                              half * cs + c0 + w],
                                    start=(half == 0),
                                    stop=False,
                                )

                # close out each count chunk region and copy it to SBUF
                # immediately so the beta/gamma chain starts while later
                # chunks still reduce
                for c0, w in _chunks(cs):
                    for half in (0, 1):
                        nc.tensor.matmul(
                            out=cnt_psum[0:1, c0:c0 + w],
                            lhsT=ones_bf[:],
                            rhs=acc_c[:, half * cs + c0:half * cs + c0 + w],
                            start=False,
                            stop=(half == 1),
                        )
                    nc.scalar.copy(
                        out=cnt_row[0:1, c0:c0 + w],
                        in_=cnt_psum[0:1, c0:c0 + w],
                    )
                nc.vector.tensor_copy(out=st_sb[:], in_=st_psum[:])

            # ---------------- phase 3: elementwise update, all local -------
            # counts for all nt3 class tiles land as columns of one [P, nt3]
            # PSUM tile, so beta/gamma come from 5 batched DVE ops; the
            # beta*center products run per-tile on the scalar engine and
            # gpsimd, overlapping the PE transposes of S.T.
            with (
                tc.tile_pool(name="p3", bufs=2) as p3,
                tc.tile_pool(name="psum3", bufs=1, space="PSUM") as psum3,
            ):
                cnt_all = psum3.tile([P, nt3], f32, name="cnt_all", space="PSUM")
                for tt in range(nt3):
                    w = min(P, cs - tt * P)
                    nc.tensor.transpose(
                        out=cnt_all[0:w, tt:tt + 1],
                        in_=cnt_row[0:1, tt * P:tt * P + w],
                        identity=identity[0:1, 0:1],
                    )
                den = p3.tile([P, nt3], f32, name="den", tag="den", bufs=1)
                nc.vector.tensor_scalar_add(out=den[:], in0=cnt_all[:], scalar1=1.0)
                rec = p3.tile([P, nt3], f32, name="rec", tag="rec", bufs=1)
                nc.vector.reciprocal(out=rec[:], in_=den[:])
                gam = p3.tile([P, nt3], f32, name="gam", tag="gam", bufs=1)
                nc.vector.tensor_scalar_mul(out=gam[:], in0=rec[:], scalar1=0.5)
                bet = p3.tile([P, nt3], f32, name="bet", tag="bet", bufs=1)
                nc.vector.tensor_tensor(
                    out=bet[:], in0=cnt_all[:], in1=rec[:], op=mult
                )
                nc.vector.tensor_scalar(
                    out=bet[:], in0=bet[:],
                    scalar1=-0.5, scalar2=1.0, op0=mult, op1=add,
                )

                for tt in range(nt3):
                    w = min(P, cs - tt * P)
                    # o1 = beta * center with per-partition scale, split
                    # across the scalar engine and gpsimd so both halves run
                    # while the PE transposes S.T tiles
                    if tt % 2 == 0:
                        nc.scalar.mul(
                            out=o1_all[0:w, tt * d:tt * d + d],
                            in_=ctr_sb[0:w, tt * d:tt * d + d],
                            mul=bet[0:w, tt:tt + 1],
                        )
                    else:
                        nc.gpsimd.tensor_scalar_mul(
                            out=o1_all[0:w, tt * d:tt * d + d],
                            in0=ctr_sb[0:w, tt * d:tt * d + d],
                            scalar1=bet[0:w, tt:tt + 1],
                        )
                    trp = psum3.tile([P, d], f32, name=f"trp_{tt}", tag="trp",
                                     bufs=4, space="PSUM")
                    nc.tensor.transpose(
                        out=trp[0:w, 0:d],
                        in_=st_sb[:, tt * P:tt * P + w],
                        identity=identity[:, 0:d],
                    )
                    nc.vector.scalar_tensor_tensor(
                        out=ou_all[0:w, tt * d:tt * d + d], in0=trp[0:w, 0:d],
                        scalar=gam[0:w, tt:tt + 1],
                        in1=o1_all[0:w, tt * d:tt * d + d], op0=mult, op1=add,
                    )
                    if tt == nt3 // 2 - 1:
                        # first half of the shard is final: overlap its store
                        nc.sync.dma_start(
                            out=out[:, 0:(nt3 // 2) * d],
                            in_=ou_all[:, 0:(nt3 // 2) * d],
                        )
                nc.scalar.dma_start(
                    out=out[:, (nt3 // 2) * d:nt3 * d],
                    in_=ou_all[:, (nt3 // 2) * d:nt3 * d],
                )

    nc.compile()
    return nc


_PROGRAM = None
LAST_RESULTS = None  # BassKernelResults from the most recent run (for test.py)


def _get_program():
    global _PROGRAM
    if _PROGRAM is None:
        _PROGRAM = build_program()
    return _PROGRAM


def kernel(embeded_preds, labels, center):
    from concourse.bass_utils import run_bass_kernel_spmd

    global LAST_RESULTS
    preds = np.ascontiguousarray(np.asarray(embeded_preds, dtype=np.float32))
    lab = np.ascontiguousarray(np.asarray(labels, dtype=np.float32))
    ctr = np.ascontiguousarray(np.asarray(center, dtype=np.float32))
    assert preds.shape == (N, D) and lab.shape == (N, C) and ctr.shape == (C, D)

    nc = _get_program()
    preds_nat = preds.reshape(P, KT * D)  # free view; bytes unchanged

    def permute_center(cj):
        # [cs, d] -> [P, nt3*d] with [p, tt*d + j] = cj[tt*P + p, j]
        cpad = np.zeros((NT3 * P, D), dtype=np.float32)
        cpad[:cj.shape[0]] = cj
        return np.ascontiguousarray(
            cpad.reshape(NT3, P, D).transpose(1, 0, 2).reshape(P, NT3 * D)
        )

    in_maps = [
        {
            "preds": preds_nat,
            "labels": np.ascontiguousarray(lab[:, j * CS:(j + 1) * CS])
                .reshape(N // 2, 2 * CS),
            "center": permute_center(ctr[j * CS:(j + 1) * CS]),
        }
        for j in range(NCORES)
    ]
    res = run_bass_kernel_spmd(nc, in_maps, core_ids=list(range(NCORES)))
    LAST_RESULTS = res

    def unpermute_out(oj):
        # [P, nt3*d] -> [cs, d]: inverse of permute_center
        return oj.reshape(P, NT3, D).transpose(1, 0, 2).reshape(NT3 * P, D)[:CS]

    return np.concatenate(
        [unpermute_out(res.results[j]["out"]) for j in range(NCORES)], axis=0
    )
